# revision 38
# baseline (speedup 1.0000x reference)
"""Trainium2 Bass kernel for a dense pre-LN transformer block (B=2, T=2048, C=1024, H=16).

Sharding: zero-collective sequence parallelism over 8 cores. Core c handles
batch b=c//4 and query tiles {r, 7-r, 8+r, 15-r} (r=c%4, 128 rows each): it
computes LN1 on the full k/v of its batch, all 16 attention heads for its 512
query rows, and the attention projection + full MLP for those rows. The
program is identical on every core; per-core causal masks are input data.

v2 layout of work per engine (vs v1 baseline at ~494us):
 - PE warm-up dummies at t=0 so HAM unthrottles before real matmuls.
 - LN1 stats (DVE bn_stats) in batches; rstd = exp(-0.5*ln(var+eps)) on ACT
   so the whole kernel uses one table set (natural_log_exp) until gelu.
 - LN normalize on DVE tensor_scalar (ACT does only exp + denominators).
 - ln1 w/b on the k side folded into the q operand (w^2, b*w); per-query
   constants cancel in softmax. kT evacuation is a plain bf16 copy.
 - Attention processes head pairs (2ct, 2ct+1): QK matmuls row-packed via
   tile_position (concurrent 64-row groups), off-chunks restricted to the
   128 live columns. Prep of k8-15/v8-15 interleaved between head pairs.
 - cproj/proj bias+residual fused into one scalar_tensor_tensor each.
 - Output stays C-major; host de-transposes (no PE output transposes).
"""

import sys

sys.path.insert(0, "/opt/trn_rl_repo")

import numpy as np
import ml_dtypes

import concourse.bass as bass
import concourse.bacc as bacc
import concourse.mybir as mybir
import concourse.tile as tile
from concourse.bass_utils import run_bass_kernel_spmd

F32 = mybir.dt.float32
BF16 = mybir.dt.bfloat16
AF = mybir.ActivationFunctionType
ALU = mybir.AluOpType

B, T, C, H, D = 2, 2048, 1024, 16, 64
NT = T // 128          # 16 key tiles
NC = C // 128          # 8 channel tiles
NF = 4 * C // 128      # 32 fc tiles
NSLOT = 4              # query tiles per core
N_CORES = 8
EPS = 1e-5
SCALE = 1.0 / 8.0      # 1/sqrt(D)

_CACHE = {}


def build():
    nc = bacc.Bacc("TRN2", target_bir_lowering=False, debug=False,
                   num_devices=N_CORES)

    q_d = nc.dram_tensor("q_s", [NSLOT, 128, C], F32, kind="ExternalInput")
    k_d = nc.dram_tensor("k_f", [NT, 128, C], BF16, kind="ExternalInput")
    v_d = nc.dram_tensor("v_f", [NT, 128, C], BF16, kind="ExternalInput")
    mask_d = nc.dram_tensor("mask", [128, NSLOT, 4, 128], BF16, kind="ExternalInput")
    cpw_d = nc.dram_tensor("cpw_t", [C, C], BF16, kind="ExternalInput")
    fcw_d = nc.dram_tensor("fcw_t", [C, 4 * C], BF16, kind="ExternalInput")
    pjw_d = nc.dram_tensor("pjw_t", [4 * C, C], BF16, kind="ExternalInput")
    # vecs cols: 0 ln1_w, 1 ln1_b, 2 attn_proj_b, 3 proj_b, 4 w1^2, 5 b1*w1
    vecs_d = nc.dram_tensor("vecs", [C, 6], F32, kind="ExternalInput")
    selw_d = nc.dram_tensor("selw1", [2, NC, 128], BF16, kind="ExternalInput")
    w2f_d = nc.dram_tensor("w2b2f", [2, C], F32, kind="ExternalInput")
    fcb_d = nc.dram_tensor("fcb", [4 * C], F32, kind="ExternalInput")
    out_d = nc.dram_tensor("out", [NC, 128, 512], F32, kind="ExternalOutput")

    NLN = NSLOT + 2 * NT   # 36 LN stat rows: q 0-3, k 4-19, v 20-35

    with tile.TileContext(nc) as tc:
      with tc.tile_pool(name="pg", bufs=1) as pg:
        # ---- constants / long-lived vectors ----
        vecs = pg.tile([128, NC, 6], F32)
        nc.sync.dma_start(vecs[:], vecs_d.ap().rearrange("(ct p) v -> p ct v", p=128))
        selw1 = pg.tile([2, NC, 128], BF16)
        nc.sync.dma_start(selw1[:], selw_d.ap())
        fcb = pg.tile([128, NF], F32)
        nc.sync.dma_start(fcb[:], fcb_d.ap().rearrange("(ft p) -> p ft", p=128))

        ones_sb = pg.tile([128, 128], F32)
        nc.gpsimd.memset(ones_sb[:], 1.0)
        ident = pg.tile([128, 128], F32)
        nc.gpsimd.affine_select(ident[:], ones_sb[:], [[1, 128]], ALU.is_equal,
                                0.0, channel_multiplier=-1)
        ones_bf = pg.tile([128, 1], BF16)
        nc.gpsimd.memset(ones_bf[:], 1.0)
        ones128_bf = pg.tile([128, 128], BF16)
        nc.gpsimd.memset(ones128_bf[:], 1.0)
        ident_bf = pg.tile([128, 128], BF16)
        nc.gpsimd.affine_select(ident_bf[:], ones128_bf[:], [[1, 128]], ALU.is_equal,
                                0.0, channel_multiplier=-1)
        ones_bcol = pg.tile([1, 128], BF16)
        nc.gpsimd.memset(ones_bcol[:], 1.0)

        ln1w = lambda ct: vecs[:, ct, 0:1]
        ln1b = lambda ct: vecs[:, ct, 1:2]
        apb = lambda ct: vecs[:, ct, 2:3]
        pjb = lambda ct: vecs[:, ct, 3:4]
        w1sq = lambda ct: vecs[:, ct, 4:5]
        b1w1 = lambda ct: vecs[:, ct, 5:6]

        # ---- cross-phase tensors ----
        qT = pg.tile([128, NC, 512], F32)      # LN1(q)^T with w,b (residual)
        qT_bf = pg.tile([128, NC, 512], BF16)  # LN1(q)^T with w^2, b*w (QK rhs)
        xT = pg.tile([128, NC, 512], F32)      # attn residual output (C-major)
        midr = pg.tile([128, NF, 256], BF16)   # raw fc output, A half (pre-gelu)
        w2f = pg.tile([128, NC, 2], F32)       # ln2 w,b per channel
        nc.sync.dma_start(w2f[:, :, 0:1],
                          w2f_d.ap()[0:1, :].rearrange("k (ct p) -> p ct k", p=128))
        nc.sync.dma_start(w2f[:, :, 1:2],
                          w2f_d.ap()[1:2, :].rearrange("k (ct p) -> p ct k", p=128))

        py_cm = tc.tile_pool(name="py", bufs=1)
        py = py_cm.__enter__()
        yT_all = py.tile([128, NC, 512], F32)  # raw attention out
        s_all = py.tile([H, 512], F32)         # softmax denominators

        with tc.tile_pool(name="pa", bufs=1) as pa:
            kT = pa.tile([128, NC, T], BF16)        # LN1(k)^T, no w/b
            v_ext = pa.tile([128, NT, H, 65], BF16)  # LN1(v) + ones col
            masks = pa.tile([128, NSLOT, 4, 128], BF16)
            nc.sync.dma_start(masks[:], mask_d.ap())

            aggr_all = pa.tile([128, NLN, 2], F32)
            rstd_all = pa.tile([128, NLN], F32)
            nmr_all = pa.tile([128, NLN], F32)
            lnt = pa.tile([128, NLN], F32)     # scratch for veps/ln

            with (
                tc.tile_pool(name="paw", bufs=2) as aw,
                tc.tile_pool(name="pst", bufs=1) as stp,
                tc.tile_pool(name="pap", bufs=2, space="PSUM") as aps,
                tc.tile_pool(name="pyp", bufs=2, space="PSUM") as pyp,
            ):
                ptp_cm = tc.tile_pool(name="ptp", bufs=2, space="PSUM")
                ptp = ptp_cm.__enter__()
                pl_cm = tc.tile_pool(name="pln", bufs=2)
                pl = pl_cm.__enter__()
                plz_cm = tc.tile_pool(name="plz", bufs=1)
                plz = plz_cm.__enter__()
                # ---- t=0: warm the PE / preload the ACT table ----
                # Real matmuls (transpose-mode does NOT count for HAM warm-up):
                # ~4us dense burst to force K=8/8 early.
                wps = ptp.tile([128, 4, 128], F32, tag="tp")
                for _ in range(40):
                    nc.tensor.matmul(wps[:, 0, :], ident_bf[:], ones128_bf[:],
                                     skip_group_check=True)
                warm_act = pl.tile([128, 1], F32, tag="wact")
                nc.scalar.activation(warm_act[:], ones_sb[:, 0:1], AF.Ln)

                # Tiny dep-paced matmuls sprinkled through DVE-heavy stretches
                # so no HAM MID window sees an idle PE (else it re-throttles).
                def ham_tick(rhs_ap, lhsT_ap, n):
                    dps = ptp.tile([1, 64], F32, tag="tp")
                    nc.tensor.matmul(dps[0:1, 0:n], lhsT_ap, rhs_ap,
                                     skip_group_check=True)

                # ---- LN helpers ----
                def stats_one(src_d, tt, idx, dt_in, tick=False):
                    x_in = pl.tile([128, C], dt_in,
                                   tag="ln_in" + ("b" if dt_in == BF16 else ""))
                    nc.sync.dma_start(x_in[:], src_d.ap()[tt])
                    stats = pl.tile([128, 2, 6], F32, tag="lns")
                    nc.vector.bn_stats(stats[:, 0, :], x_in[:, 0:512])
                    nc.vector.bn_stats(stats[:, 1, :], x_in[:, 512:1024])
                    nc.vector.bn_aggr(aggr_all[:, idx, :], stats[:])
                    if tick:
                        ham_tick(aggr_all[:, idx, :], ones_sb[:, 0:1], 2)

                def rstd_batch(i0, i1):
                    # rstd = exp(-0.5 * ln(var+eps)); nmr = -mu * rstd
                    nc.vector.tensor_scalar(lnt[:, i0:i1], aggr_all[:, i0:i1, 1],
                                            EPS, None, ALU.add)
                    nc.scalar.activation(lnt[:, i0:i1], lnt[:, i0:i1], AF.Ln)
                    nc.scalar.activation(rstd_all[:, i0:i1], lnt[:, i0:i1],
                                         AF.Exp, scale=-0.5)
                    nc.vector.scalar_tensor_tensor(
                        nmr_all[:, i0:i1], aggr_all[:, i0:i1, 0], -1.0,
                        rstd_all[:, i0:i1], ALU.mult, ALU.mult)

                # q: normalize (DVE) -> transpose (PE) -> w,b + w2,bw (ACT)
                def q_prep(tick=False):
                    zs = []
                    for gi in range(NSLOT):
                        x_in = pl.tile([128, C], F32, tag="ln_in")
                        nc.sync.dma_start(x_in[:], q_d.ap()[gi])
                        z = plz.tile([128, C], F32, tag=f"z{gi}")
                        nc.vector.tensor_scalar(z[:], x_in[:],
                                                rstd_all[:, gi:gi + 1],
                                                nmr_all[:, gi:gi + 1],
                                                ALU.mult, ALU.add)
                        zs.append(z)
                    for ct in range(NC):
                        ps = ptp.tile([128, 4, 128], F32, tag="tp")
                        for gi in range(4):
                            nc.tensor.transpose(ps[:, gi, :],
                                                zs[gi][:, ct * 128:(ct + 1) * 128],
                                                ident[:])
                        nc.scalar.activation(qT[:, ct, :], ps[:], AF.Identity,
                                             bias=ln1b(ct), scale=ln1w(ct))
                        nc.scalar.activation(qT_bf[:, ct, :], ps[:], AF.Identity,
                                             bias=b1w1(ct), scale=w1sq(ct))
                        if tick:
                            ham_tick(zs[0][:, ct * 128:ct * 128 + 64],
                                     ones_sb[:, 0:1], 64)

                # k group of 4: normalize -> transpose -> plain copy into kT
                def k_grp(tts, idx0, tick=False):
                    zs = []
                    for gi, tt in enumerate(tts):
                        x_in = pl.tile([128, C], BF16, tag="ln_inb")
                        nc.sync.dma_start(x_in[:], k_d.ap()[tt])
                        z = plz.tile([128, C], BF16, tag=f"z{gi}")
                        i = idx0 + gi
                        nc.vector.tensor_scalar(z[:], x_in[:],
                                                rstd_all[:, i:i + 1],
                                                nmr_all[:, i:i + 1],
                                                ALU.mult, ALU.add)
                        zs.append(z)
                    dst_off = (tts[0] // 4) * 512
                    for ct in range(NC):
                        ps = ptp.tile([128, 4, 128], F32, tag="tp")
                        pv = ps[:].bitcast(BF16)[:, :, 0:128]
                        for gi in range(4):
                            nc.tensor.transpose(pv[:, gi, :],
                                                zs[gi][:, ct * 128:(ct + 1) * 128],
                                                ident_bf[:])
                        nc.scalar.copy(kT[:, ct, dst_off:dst_off + 512], pv[:])
                        if tick:
                            ham_tick(zs[0][:, ct * 128:ct * 128 + 64],
                                     ones_bf[:], 64)

                def v_grp(tts, tick=False):
                    for tt in tts:
                        x_in = pl.tile([128, C], BF16, tag="ln_inb")
                        nc.sync.dma_start(x_in[:], v_d.ap()[tt])
                        i = NSLOT + NT + tt
                        nc.gpsimd.memset(v_ext[:, tt, :, 64:65], 1.0)
                        nc.gpsimd.tensor_scalar(
                            v_ext[:, tt, :, 0:64],
                            x_in[:].rearrange("p (h d) -> p h d", h=H),
                            rstd_all[:, i:i + 1], nmr_all[:, i:i + 1],
                            ALU.mult, ALU.add)
                        if tick:
                            ham_tick(x_in[:, 0:64], ones_bf[:], 64)

                # ---- attention head-pair step ----
                def attn_pair(hp, np_, c0, c1, nfrom):
                    yp = pyp.tile([65, 2, 256], F32, tag="yp")
                    for ch in range(np_ // 4):
                        pbase = ch * 4
                        off = 0 if pbase < nfrom else 128
                        w = 256 - off
                        # alternate row groups so the two heads' QK matmuls
                        # execute concurrently in the PE array
                        sc0 = aps.tile([128, 4, 256], F32, tag="sc")
                        sc1 = aps.tile([128, 4, 256], F32, tag="sc")
                        scs = [sc0, sc1]
                        for pc in range(4):
                            p = pbase + pc
                            for hpi in range(2):
                                sel = hpi * 64
                                nc.tensor.matmul(
                                    scs[hpi][:, pc, off:256],
                                    kT[sel:sel + 64, hp, p * 128:(p + 1) * 128],
                                    qT_bf[sel:sel + 64, hp, c0 + off:c1],
                                    tile_position=(sel, 0),
                                    skip_group_check=True)
                        atts = []
                        for hpi in range(2):
                            att = aw.tile([128, 4, 256], BF16, tag="att")
                            nc.scalar.activation(att[:, :, off:256],
                                                 scs[hpi][:, :, off:256],
                                                 AF.Exp, scale=SCALE)
                            atts.append(att)
                        for i in range(NSLOT):
                            if c0 <= i * 128 < c1 and i * 4 == pbase:
                                acol = i * 128 - c0
                                for hpi in range(2):
                                    nc.vector.tensor_tensor(
                                        atts[hpi][:, :, acol:acol + 128],
                                        atts[hpi][:, :, acol:acol + 128],
                                        masks[:, i, :, :],
                                        ALU.mult)
                        for hpi in range(2):
                            for pc in range(4):
                                p = pbase + pc
                                # yp holds both heads in one PSUM bank: only the
                                # bank's first matmul may use start=True (it
                                # clears has_written for the WHOLE bank).
                                nc.tensor.matmul(
                                    yp[:, hpi, off:256],
                                    v_ext[:, p, 2 * hp + hpi, :],
                                    atts[hpi][:, pc, off:256],
                                    start=(p == 0 and hpi == 0),
                                    stop=(p == np_ - 1 and hpi == 1),
                                    skip_group_check=True)
                    st = stp.tile([65, 2, 256], F32, tag="sty")
                    nc.vector.tensor_copy(st[:], yp[:])
                    nc.vector.tensor_copy(yT_all[0:64, hp, c0:c1], st[0:64, 0, :])
                    nc.vector.tensor_copy(yT_all[64:128, hp, c0:c1], st[0:64, 1, :])
                    for hpi in range(2):
                        nc.sync.dma_start(s_all[2 * hp + hpi:2 * hp + hpi + 1, c0:c1],
                                          st[64:65, hpi, :])

                # ================= prologue: batch 0 =================
                for i in range(NSLOT):
                    stats_one(q_d, i, i, F32, tick=True)
                for tt in range(8):
                    stats_one(k_d, tt, NSLOT + tt, BF16, tick=True)
                rstd_batch(0, 12)
                q_prep(tick=True)
                k_grp(range(0, 4), NSLOT + 0, tick=True)
                k_grp(range(4, 8), NSLOT + 4, tick=True)
                for tt in range(8):
                    stats_one(v_d, tt, NSLOT + NT + tt, BF16, tick=True)
                rstd_batch(20, 28)
                v_grp(range(0, 8), tick=True)

                # prep-B tasks interleaved into attention-A below
                def pb_stats_k(a, b):
                    def f():
                        for tt in range(a, b):
                            stats_one(k_d, tt, NSLOT + tt, BF16)
                    return f

                def pb_stats_v(a, b):
                    def f():
                        for tt in range(a, b):
                            stats_one(v_d, tt, NSLOT + NT + tt, BF16)
                    return f

                prep_b = [
                    pb_stats_k(8, 10), pb_stats_k(10, 12),
                    pb_stats_k(12, 14), pb_stats_k(14, 16),
                    lambda: rstd_batch(12, 20),
                    lambda: k_grp(range(8, 12), NSLOT + 8),
                    lambda: k_grp(range(12, 16), NSLOT + 12),
                    pb_stats_v(8, 12), pb_stats_v(12, 16),
                    lambda: rstd_batch(28, 36),
                    lambda: v_grp(range(8, 12)),
                    lambda: v_grp(range(12, 16)),
                ]

                # ============ attention A (np8, cols 0:256) ============
                for hp in range(8):
                    attn_pair(hp, 8, 0, 256, 4)
                    if hp < 4:
                        prep_b[hp]()
                    else:
                        prep_b[4 + (hp - 4) * 2]()
                        prep_b[5 + (hp - 4) * 2]()
                plz_cm.__exit__(None, None, None)
                pl_cm.__exit__(None, None, None)
                ptp_cm.__exit__(None, None, None)

                # ===== stage 2: attention B (np16, cols 256:512) overlapped
                # with y-scale/cproj/LN2/fc of the A half (cols 0:256).
                # Dense fc chains also keep the PE HAM-warm during attention.
                pyc_cm = tc.tile_pool(name="pyc", bufs=1)
                pyc = pyc_cm.__enter__()
                cpwT = pyc.tile([128, NC, C], BF16)
                nc.sync.dma_start(cpwT[:],
                                  cpw_d.ap().rearrange("(ct p) o -> p ct o", p=128))
                yscA = py.tile([128, NC, 256], BF16)
                z2A = py.tile([128, NC, 256], BF16)
                zAsb = py.tile([128, 256], F32)
                zBsb = py.tile([128, 256], F32)
                srec_b = py.tile([H, 512], BF16)
                srec2 = py.tile([2, NC, 512], BF16)

                with (
                    tc.tile_pool(name="p2w", bufs=2) as w2p,
                    tc.tile_pool(name="p2ps", bufs=2, space="PSUM") as mps2,
                ):
                    def recipA():
                        nc.vector.reciprocal(s_all[:, 0:256], s_all[:, 0:256])
                        nc.vector.tensor_copy(srec_b[:, 0:256], s_all[:, 0:256])
                        for hp2 in range(NC):
                            nc.sync.dma_start(srec2[:, hp2, 0:256],
                                              srec_b[2 * hp2:2 * hp2 + 2, 0:256])

                    def yscA_task(cts):
                        for ct in cts:
                            rb = mps2.tile([128, 256], F32, tag="macc")
                            nc.tensor.matmul(rb[:], selw1[:, ct, :],
                                             srec2[:, ct, 0:256],
                                             skip_group_check=True)
                            t1 = w2p.tile([128, 256], F32, tag="t1")
                            nc.vector.scalar_tensor_tensor(
                                t1[:], yT_all[:, ct, 0:256], 1.0, rb[:],
                                ALU.mult, ALU.mult)
                            nc.vector.tensor_scalar(yscA[:, ct, :], t1[:], 1.0,
                                                    ln1b(ct), ALU.mult, ALU.add)

                    def cprojA_task(ots):
                        for ot in ots:
                            cp = mps2.tile([128, 256], F32, tag="macc")
                            for ct in range(NC):
                                nc.tensor.matmul(
                                    cp[:], cpwT[:, ct, ot * 128:(ot + 1) * 128],
                                    yscA[:, ct, :], start=(ct == 0),
                                    stop=(ct == NC - 1))
                            nc.vector.scalar_tensor_tensor(
                                xT[:, ot, 0:256], cp[:], apb(ot),
                                qT[:, ot, 0:256], ALU.add, ALU.add)

                    def ln2A_task():
                        s1 = mps2.tile([1, 256], F32, tag="macc")
                        for ct in range(NC):
                            nc.tensor.matmul(s1[:], ones_sb[:, 0:1],
                                             xT[:, ct, 0:256],
                                             start=(ct == 0), stop=(ct == NC - 1),
                                             skip_group_check=True)
                        s2 = mps2.tile([1, 256], F32, tag="macc")
                        for ct in range(NC):
                            sq = w2p.tile([128, 256], BF16, tag="sq")
                            nc.vector.tensor_tensor(sq[:], xT[:, ct, 0:256],
                                                    xT[:, ct, 0:256], ALU.mult)
                            nc.tensor.matmul(s2[:], ones_bf[:], sq[:],
                                             start=(ct == 0), stop=(ct == NC - 1),
                                             skip_group_check=True)
                        mu = py.tile([1, 256], F32)
                        nc.vector.tensor_scalar(mu[:], s1[:], 1.0 / C, None,
                                                ALU.mult)
                        var = py.tile([1, 256], F32)
                        nc.vector.tensor_scalar(var[:], s2[:], 1.0 / C, EPS,
                                                ALU.mult, ALU.add)
                        nmu2 = py.tile([1, 256], F32)
                        nc.vector.scalar_tensor_tensor(nmu2[:], mu[:], -1.0,
                                                       mu[:], ALU.mult, ALU.mult)
                        nc.vector.tensor_tensor(var[:], var[:], nmu2[:], ALU.add)
                        rstd2 = py.tile([1, 256], F32)
                        nc.scalar.activation(var[:], var[:], AF.Ln)
                        nc.scalar.activation(rstd2[:], var[:], AF.Exp, scale=-0.5)
                        nmr2 = py.tile([1, 256], F32)
                        nc.vector.scalar_tensor_tensor(nmr2[:], mu[:], -1.0,
                                                       rstd2[:], ALU.mult, ALU.mult)
                        rstd2b = py.tile([1, 256], BF16)
                        nc.vector.tensor_copy(rstd2b[:], rstd2[:])
                        nmr2b = py.tile([1, 256], BF16)
                        nc.vector.tensor_copy(nmr2b[:], nmr2[:])
                        zA = mps2.tile([128, 256], F32, tag="macc")
                        nc.tensor.matmul(zA[:], ones_bcol[:], rstd2b[:],
                                         skip_group_check=True)
                        nc.vector.tensor_copy(zAsb[:], zA[:])
                        zB = mps2.tile([128, 256], F32, tag="macc")
                        nc.tensor.matmul(zB[:], ones_bcol[:], nmr2b[:],
                                         skip_group_check=True)
                        nc.vector.tensor_copy(zBsb[:], zB[:])

                    def z2A_task(cts):
                        for ct in cts:
                            t1 = w2p.tile([128, 256], F32, tag="t1")
                            nc.vector.scalar_tensor_tensor(
                                t1[:], xT[:, ct, 0:256], 1.0, zAsb[:],
                                ALU.mult, ALU.mult)
                            nc.vector.scalar_tensor_tensor(
                                t1[:], t1[:], 1.0, zBsb[:], ALU.mult, ALU.add)
                            nc.vector.tensor_scalar(z2A[:, ct, :], t1[:],
                                                    w2f[:, ct, 0:1],
                                                    w2f[:, ct, 1:2],
                                                    ALU.mult, ALU.add)

                    def fcA_task(fts):
                        for ft in fts:
                            fw = w2p.tile([128, NC, 128], BF16, tag="fw")
                            nc.sync.dma_start(
                                fw[:], fcw_d.ap()[:, ft * 128:(ft + 1) * 128]
                                .rearrange("(ct p) f -> p ct f", p=128))
                            facc = mps2.tile([128, 256], F32, tag="macc")
                            for ct in range(NC):
                                nc.tensor.matmul(facc[:], fw[:, ct, :],
                                                 z2A[:, ct, :], start=(ct == 0),
                                                 stop=(ct == NC - 1))
                            nc.vector.tensor_copy(midr[:, ft, :], facc[:])

                    mlpA = [
                        lambda: (recipA(), yscA_task(range(0, 4))),
                        lambda: (yscA_task(range(4, 8)), cprojA_task(range(0, 2))),
                        lambda: cprojA_task(range(2, 8)),
                        lambda: ln2A_task(),
                        lambda: z2A_task(range(0, 8)),
                        lambda: fcA_task(range(0, 10)),
                        lambda: fcA_task(range(10, 21)),
                        lambda: fcA_task(range(21, 32)),
                    ]
                    for hp in range(8):
                        attn_pair(hp, 16, 256, 512, 12)
                        mlpA[hp]()

                # B-half softmax reciprocals
                nc.vector.reciprocal(s_all[:, 256:512], s_all[:, 256:512])
                nc.vector.tensor_copy(srec_b[:, 256:512], s_all[:, 256:512])
                for hp in range(NC):
                    nc.sync.dma_start(srec2[:, hp, 256:512],
                                      srec_b[2 * hp:2 * hp + 2, 256:512])

                # ---- y-scale/cproj of B half ----
                with (
                    tc.tile_pool(name="pcw", bufs=3) as cw,
                    tc.tile_pool(name="pcps", bufs=1, space="PSUM") as cps,
                ):
                    yscB = pyc.tile([128, NC, 256], BF16)
                    for ct in range(NC):
                        rb = cps.tile([128, 256], F32, tag="rb")
                        nc.tensor.matmul(rb[:], selw1[:, ct, :],
                                         srec2[:, ct, 256:512],
                                         skip_group_check=True)
                        t1 = cw.tile([128, 256], F32, tag="yt1")
                        nc.vector.scalar_tensor_tensor(
                            t1[:], yT_all[:, ct, 256:512], 1.0, rb[:],
                            ALU.mult, ALU.mult)
                        nc.vector.tensor_scalar(yscB[:, ct, :], t1[:], 1.0,
                                                ln1b(ct), ALU.mult, ALU.add)
                    for ot in range(NC):
                        pj = cps.tile([128, 256], F32, tag="cp")
                        for ct in range(NC):
                            nc.tensor.matmul(
                                pj[:], cpwT[:, ct, ot * 128:(ot + 1) * 128],
                                yscB[:, ct, :], start=(ct == 0),
                                stop=(ct == NC - 1))
                        nc.vector.scalar_tensor_tensor(
                            xT[:, ot, 256:512], pj[:], apb(ot),
                            qT[:, ot, 256:512], ALU.add, ALU.add)
                pyc_cm.__exit__(None, None, None)

        py_cm.__exit__(None, None, None)

        # ================= LN2 + MLP =================
        with (
            tc.tile_pool(name="pm", bufs=1) as pm,
            tc.tile_pool(name="pmw", bufs=3) as mw,
            tc.tile_pool(name="pmo", bufs=2) as mo,
            tc.tile_pool(name="pms", bufs=1, space="PSUM") as mps,
            tc.tile_pool(name="pma", bufs=2, space="PSUM") as mac,
        ):
            # LN2 stats for the B half (cols 256:512)
            s1 = mps.tile([1, 256], F32, tag="s1")
            s2 = mps.tile([1, 256], F32, tag="s2")
            for ct in range(NC):
                nc.tensor.matmul(s1[:], ones_sb[:, 0:1], xT[:, ct, 256:512],
                                 start=(ct == 0), stop=(ct == NC - 1),
                                 skip_group_check=True)
            for ct in range(NC):
                sq = mw.tile([128, 256], BF16, tag="sq")
                nc.vector.tensor_tensor(sq[:], xT[:, ct, 256:512],
                                        xT[:, ct, 256:512], ALU.mult)
                nc.tensor.matmul(s2[:], ones_bf[:], sq[:],
                                 start=(ct == 0), stop=(ct == NC - 1),
                                 skip_group_check=True)
            mu = pm.tile([1, 256], F32)
            nc.vector.tensor_scalar(mu[:], s1[:], 1.0 / C, None, ALU.mult)
            var = pm.tile([1, 256], F32)
            nc.vector.tensor_scalar(var[:], s2[:], 1.0 / C, EPS, ALU.mult, ALU.add)
            negmu2 = pm.tile([1, 256], F32)
            nc.vector.scalar_tensor_tensor(negmu2[:], mu[:], -1.0, mu[:],
                                           ALU.mult, ALU.mult)
            nc.vector.tensor_tensor(var[:], var[:], negmu2[:], ALU.add)
            rstd2 = pm.tile([1, 256], F32)
            nc.scalar.activation(var[:], var[:], AF.Ln)
            nc.scalar.activation(rstd2[:], var[:], AF.Exp, scale=-0.5)
            nmr2 = pm.tile([1, 256], F32)
            nc.vector.scalar_tensor_tensor(nmr2[:], mu[:], -1.0, rstd2[:],
                                           ALU.mult, ALU.mult)
            rstd2b = pm.tile([1, 256], BF16)
            nc.vector.tensor_copy(rstd2b[:], rstd2[:])
            nmr2b = pm.tile([1, 256], BF16)
            nc.vector.tensor_copy(nmr2b[:], nmr2[:])

            zA = mps.tile([128, 256], F32, tag="zA")
            zB = mps.tile([128, 256], F32, tag="zB")
            nc.tensor.matmul(zA[:], ones_bcol[:], rstd2b[:], skip_group_check=True)
            nc.tensor.matmul(zB[:], ones_bcol[:], nmr2b[:], skip_group_check=True)

            # z2 (B half) = ((x * zA + zB) * w2[c] + b2[c]), bf16
            z2 = pm.tile([128, NC, 256], BF16)
            for ct in range(NC):
                t1 = mw.tile([128, 256], F32, tag="z2t")
                nc.vector.scalar_tensor_tensor(t1[:], xT[:, ct, 256:512], 1.0,
                                               zA[:], ALU.mult, ALU.mult)
                nc.vector.scalar_tensor_tensor(t1[:], t1[:], 1.0, zB[:],
                                               ALU.mult, ALU.add)
                nc.vector.tensor_scalar(z2[:, ct, :], t1[:], w2f[:, ct, 0:1],
                                        w2f[:, ct, 1:2], ALU.mult, ALU.add)

            # gelu of A half (from staged raw fc) + fc/gelu of B half -> mid
            mid = pm.tile([128, NF, 512], BF16)
            for ft in range(NF):
                nc.scalar.activation(mid[:, ft, 0:256], midr[:, ft, :],
                                     AF.Gelu_apprx_tanh, bias=fcb[:, ft:ft + 1])
                fw = mw.tile([128, NC, 128], BF16, tag="fw")
                nc.sync.dma_start(fw[:], fcw_d.ap()[:, ft * 128:(ft + 1) * 128]
                                  .rearrange("(ct p) f -> p ct f", p=128))
                fp = mac.tile([128, 256], F32, tag="acc")
                for ct in range(NC):
                    nc.tensor.matmul(fp[:], fw[:, ct, :], z2[:, ct, :],
                                     start=(ct == 0), stop=(ct == NC - 1))
                nc.scalar.activation(mid[:, ft, 256:512], fp[:],
                                     AF.Gelu_apprx_tanh, bias=fcb[:, ft:ft + 1])

            # proj + pjb + residual -> out (C-major; host de-transposes)
            for ot in range(NC):
                pw = mw.tile([128, NF, 128], BF16, tag="pw")
                nc.sync.dma_start(pw[:], pjw_d.ap()[:, ot * 128:(ot + 1) * 128]
                                  .rearrange("(ft p) f -> p ft f", p=128))
                pacc = mac.tile([128, 512], F32, tag="acc")
                for ft in range(NF):
                    nc.tensor.matmul(pacc[:], pw[:, ft, :], mid[:, ft, :],
                                     start=(ft == 0), stop=(ft == NF - 1))
                ob = mo.tile([128, 512], F32, tag="ob")
                nc.vector.scalar_tensor_tensor(ob[:], pacc[:], pjb(ot),
                                               xT[:, ot, :], ALU.add, ALU.add)
                nc.sync.dma_start(out_d.ap()[ot], ob[:])

    nc.compile()
    return nc


def _host_prep(inputs):
    q = np.asarray(inputs["q"], np.float32)
    k = np.asarray(inputs["k"], np.float32)
    v = np.asarray(inputs["v"], np.float32)
    cpw_t = np.ascontiguousarray(np.asarray(inputs["attn_proj_w"], np.float32).T
                                 ).astype(ml_dtypes.bfloat16)
    fcw_t = np.ascontiguousarray(np.asarray(inputs["fc_w"], np.float32).T
                                 ).astype(ml_dtypes.bfloat16)
    pjw_t = np.ascontiguousarray(np.asarray(inputs["proj_w"], np.float32).T
                                 ).astype(ml_dtypes.bfloat16)
    w1 = np.asarray(inputs["ln1_w"], np.float32)
    b1 = np.asarray(inputs["ln1_b"], np.float32)
    vecs = np.ascontiguousarray(np.stack(
        [w1, b1,
         np.asarray(inputs["attn_proj_b"], np.float32),
         np.asarray(inputs["proj_b"], np.float32),
         w1 * w1, b1 * w1], axis=1))
    # selw1[j, ct, c] = w1[ct*128+c] if head-half j matches c else 0
    selw1 = np.zeros((2, NC, 128), np.float32)
    for ct in range(NC):
        blk = w1[ct * 128:(ct + 1) * 128]
        selw1[0, ct, 0:64] = blk[0:64]
        selw1[1, ct, 64:128] = blk[64:128]
    selw1 = selw1.astype(ml_dtypes.bfloat16)
    w2b2f = np.ascontiguousarray(np.stack(
        [np.asarray(inputs["ln2_w"], np.float32),
         np.asarray(inputs["ln2_b"], np.float32)], axis=0))
    fcb = np.ascontiguousarray(np.asarray(inputs["fc_b"], np.float32))

    tri = (np.arange(128)[:, None] <= np.arange(128)[None, :])  # keep tk<=tq

    in_maps, slot_map = [], []
    for c in range(N_CORES):
        b, r = c // 4, c % 4
        slots = [r, 7 - r, 8 + r, 15 - r]
        slot_map.append((b, slots))
        qs = q[b].reshape(NT, 128, C)[slots]
        mask = np.zeros((128, NSLOT, 4, 128), np.float32)
        for i, a in enumerate(slots):
            for p4 in range(4):
                p = 4 * i + p4
                if p < a:
                    mask[:, i, p4, :] = 1.0
                elif p == a:
                    mask[:, i, p4, :] = tri
        in_maps.append({
            "q_s": np.ascontiguousarray(qs),
            "k_f": np.ascontiguousarray(k[b].reshape(NT, 128, C)).astype(ml_dtypes.bfloat16),
            "v_f": np.ascontiguousarray(v[b].reshape(NT, 128, C)).astype(ml_dtypes.bfloat16),
            "mask": mask.astype(ml_dtypes.bfloat16),
            "cpw_t": cpw_t, "fcw_t": fcw_t, "pjw_t": pjw_t,
            "vecs": vecs, "selw1": selw1, "w2b2f": w2b2f, "fcb": fcb,
        })
    return in_maps, slot_map


def kernel(**inputs):
    if "nc" not in _CACHE:
        _CACHE["nc"] = build()
    nc = _CACHE["nc"]
    in_maps, slot_map = _host_prep(inputs)
    res = run_bass_kernel_spmd(nc, in_maps, core_ids=list(range(N_CORES)))
    out = np.empty((B, T, C), np.float32)
    for c in range(N_CORES):
        b, slots = slot_map[c]
        o = res.results[c]["out"]   # [NC, 128, 512] C-major
        for i, a in enumerate(slots):
            out[b, a * 128:(a + 1) * 128, :] = (
                o[:, :, i * 128:(i + 1) * 128].transpose(2, 0, 1).reshape(128, C))
    return out
